# revision 8
# baseline (speedup 1.0000x reference)
# Multi-head attention (B=4, S=2048, D=512, H=8) on 8 Trainium2 cores.
#
# Sharding: core c = (batch b=c//2, head-group g=c%2, 4 heads each). Every core
# runs the identical program (SPMD) on its own slice; partial w_o outputs of the
# two head-groups of a batch are summed on the host (+ b_o).
#
# Device dataflow keeps every activation transposed ([feature, token]) so no
# on-device transposes are needed:
#   QT = w_q_g @ qT + b  (PE, din on partitions)        [256, 2048]
#   KT = (w_k_g/8) @ kT + b/8                            [256, 2048]
#   V  = natural [keys, dout] via lhsT = vT chunks       [2048, 4, 65] (+ones col)
#   scores^T[k, q] = K Q^T  (lhsT = KT slice)            per (qc=512, kc=128)
#     -> the two heads of a pair run as concurrent row-tiles (K=64 each, auto
#        tile_position (0,0)/(64,0)) draining to separate PSUM banks.
#   E^T = exp(scores^T + causal window mask)             ACT, merged head-pair
#   ctx^T/l = V_aug^T E^T   (m=65: row 64 = l[q])        PSUM accumulate over kc
#   out^T_partial = w_oT_g ctx^T                         [512, 2048] -> DRAM
#
# All matmul operands are bf16 (fp32r streams at ~1.2 GHz effective on the PE;
# bf16 streams at 2.4 GHz warm - 2x). PSUM stays fp32. Load/store DRAM layouts
# are pre-tiled host-side so every DMA is 128 fat contiguous descriptors.
# Scheduling: the kc loops of the causal attention are the timeline; projection
# and output-projection matmul "quanta" are interleaved into them so the PE
# stream stays dense (HAM stays warm) and the ACT (exp) stream never starves:
#   - chunk i+1's Q projection runs inside attn(i, pr=1)
#   - chunk i's K/V projections run inside attn(i, pr=0) early slots
#   - output projections run inside the ACT-heavy late chunks
import os
import sys

import numpy as np

B, S, D, H = 4, 2048, 512, 8
DK = D // H          # 64
P = 128
NCORES = 8
HG = 2               # head groups (cores per batch)
DH = D // HG         # 256 out dims per core
LH = H // HG         # 4 local heads
QCS = 512            # q/key chunk size
NQC = S // QCS       # 4
KCS = 128            # key tile size (scores psum partition dim)
NEG = -1e9

# "bf16" (2x PE stream rate) or "f32" (exact fp32 operands) for MM operands
MM_DT = os.environ.get("KERNEL_MM_DT", "bf16")

_CACHE = {}


def _import_concourse():
    for p in ("/opt/trn_rl_repo", "/root/.axon_site/_ro/trn_rl_repo"):
        if os.path.isdir(p) and p not in sys.path:
            sys.path.append(p)
    import concourse.bass as bass          # noqa: F401
    import concourse.mybir as mybir        # noqa: F401
    import concourse.tile as tile          # noqa: F401
    from concourse import bacc             # noqa: F401
    return bass, mybir, tile


def build_nc():
    """Build the (single, shared-by-all-cores) Bass program."""
    if "nc" in _CACHE:
        return _CACHE["nc"]
    bass, mybir, tile = _import_concourse()
    from concourse import bacc
    from contextlib import ExitStack

    f32 = mybir.dt.float32
    bf = mybir.dt.bfloat16 if MM_DT == "bf16" else mybir.dt.float32r
    Exp = mybir.ActivationFunctionType.Exp

    nc = bacc.Bacc("TRN2", target_bir_lowering=False, debug=False)

    qkvT = nc.dram_tensor("qkvT", [NQC, P, 3, 4, QCS], bf, kind="ExternalInput").ap()
    wqkvT = nc.dram_tensor("wqkvT", [P, 3, 4, DH], bf, kind="ExternalInput").ap()
    woTt = nc.dram_tensor("woTt", [P, 2, D], bf, kind="ExternalInput").ap()
    bqk = nc.dram_tensor("bqk", [P, 2, 2], f32, kind="ExternalInput").ap()
    bv = nc.dram_tensor("bv", [P, DH], f32, kind="ExternalInput").ap()
    mtri = nc.dram_tensor("mtri", [P, P], f32, kind="ExternalInput").ap()
    onesd = nc.dram_tensor("onesd", [P, DK], bf, kind="ExternalInput").ap()
    outTt = nc.dram_tensor("outTt", [NQC, 2, P, 2, QCS], bf, kind="ExternalOutput").ap()

    with tile.TileContext(nc) as tc, ExitStack() as ctx:
        wpool = ctx.enter_context(tc.tile_pool(name="weights", bufs=1))
        res = ctx.enter_context(tc.tile_pool(name="resident", bufs=1))
        opool = ctx.enter_context(tc.tile_pool(name="ost", bufs=4))

        # PE warm-up during the initial DMA wait: ~10 back-to-back zero matmuls
        # release the HAM clock-gate (needs ~3.4us of sustained PE activity) so
        # the first projection runs at 2.4 GHz instead of 0.65-1.2.
        wz_sb = wpool.tile([P, 5 * P], bf, tag="wz")
        nc.vector.memzero(wz_sb[:])
        with tc.tile_pool(name="warm", bufs=1, space="PSUM") as wps:
            wz_ps = wps.tile([P, QCS], f32, tag="wz")
            for _ in range(10):
                nc.tensor.matmul(
                    wz_ps[:], wz_sb[:, 0:P], wz_sb[:, P : 5 * P],
                    start=True, stop=True,
                )
            nc.vector.tensor_copy(wz_sb[:, 0:QCS], wz_ps[:])

        wqkv_sb = wpool.tile([P, 3, 4, DH], bf, tag="wqkv")
        wo_sb = wpool.tile([P, 2, D], bf, tag="wo")
        bqk_sb = wpool.tile([P, 2, 2], f32, tag="bqk")
        bv_sb = wpool.tile([P, DH], f32, tag="bv")
        mt_sb = wpool.tile([P, P], f32, tag="mtri")
        ones_sb = wpool.tile([P, DK], bf, tag="ones")

        def dma_weights():
            # scalar-queue dispatch: overlaps the sync-queue chunk loads
            nc.scalar.dma_start(wqkv_sb[:], wqkvT[:])
            nc.scalar.dma_start(bqk_sb[:], bqk[:])
            nc.scalar.dma_start(bv_sb[:], bv[:])
            nc.scalar.dma_start(mt_sb[:], mtri[:])
            nc.scalar.dma_start(ones_sb[:], onesd[:])
            nc.scalar.dma_start(wo_sb[:], woTt[:])

        wq_sb, wk_sb, wv_sb = wqkv_sb[:, 0], wqkv_sb[:, 1], wqkv_sb[:, 2]

        # per-512-chunk resident tiles -> fine-grained cross-phase deps
        QTs = [res.tile([P, 2, QCS], bf, tag=f"QT{i}", name=f"QT{i}") for i in range(NQC)]
        KTs = [res.tile([P, 2, QCS], bf, tag=f"KT{i}", name=f"KT{i}") for i in range(NQC)]
        Vgs = [
            res.tile([P, 4, LH, DK + 1], bf, tag=f"Vg{i}", name=f"Vg{i}")
            for i in range(NQC)
        ]
        CTs = [res.tile([P, 2, QCS], bf, tag=f"CT{i}", name=f"CT{i}") for i in range(NQC)]

        bv_r = bv_sb.rearrange("p (h d) -> p h d", h=LH)
        ones_r = ones_sb[:, 0 : 4 * LH].rearrange("p (a b) -> p a b", a=4)

        with (
            tc.tile_pool(name="inq", bufs=2) as qpool,
            tc.tile_pool(name="et", bufs=6) as epool,
            tc.tile_pool(name="sc", bufs=2, space="PSUM") as scp,
            tc.tile_pool(name="cx", bufs=2, space="PSUM") as cxp,
            tc.tile_pool(name="ls", bufs=2) as lpool,
            tc.tile_pool(name="cbst", bufs=2) as cbpool,
        ):
            chunk_in = {}

            def dma_chunk(fc):
                t = qpool.tile([P, 3, 4, QCS], bf, tag="qkv", name=f"qkv{fc}")
                nc.sync.dma_start(t[:], qkvT[fc])
                chunk_in[fc] = t

            def make_proj(fc):
                """8 PE quanta of 4 matmuls each, projecting chunk fc.

                Returns (q_quanta, kv_quanta): Q must finish before attn(fc)
                starts; K/V only before attn(fc,0)'s kc=4*fc / ctx slots.
                """
                st = {}

                def q_kq(which, mo):
                    ch = chunk_in[fc][:, 1 if which == "k" else 0]
                    w_sb = wk_sb if which == "k" else wq_sb
                    bsl = 1 if which == "k" else 0
                    dst = KTs[fc] if which == "k" else QTs[fc]
                    if mo == 0:
                        st[which] = scp.tile(
                            [P, 2, QCS], f32, tag="sc", name=f"ps{which}{fc}"
                        )
                    ps = st[which]
                    for c in range(4):
                        nc.tensor.matmul(
                            ps[:, mo, :], w_sb[:, c, mo * P : (mo + 1) * P],
                            ch[:, c, :], start=(c == 0), stop=(c == 3),
                        )
                    nc.vector.tensor_add(
                        dst[:, mo, :], ps[:, mo, :],
                        bqk_sb[:, bsl, mo : mo + 1].to_broadcast((P, QCS)),
                    )

                def q_v(k2, kl2):
                    vch = chunk_in[fc][:, 2]
                    if k2 == 0 and kl2 == 0:
                        nc.vector.tensor_copy(Vgs[fc][:, :, :, DK], ones_r)
                    if kl2 == 0:
                        st[f"v{k2}"] = scp.tile(
                            [P, 2, QCS], f32, tag="sc", name=f"psv{fc}{k2}"
                        )
                    psv = st[f"v{k2}"]
                    kl = k2 * 2 + kl2
                    for c in range(4):
                        nc.tensor.matmul(
                            psv[:, kl2, 0:DH],
                            vch[:, c, kl * P : (kl + 1) * P], wv_sb[:, c, :],
                            start=(c == 0), stop=(c == 3),
                        )
                    nc.vector.tensor_add(
                        Vgs[fc][:, kl, :, 0:DK],
                        psv[:, kl2, 0:DH].rearrange("p (h d) -> p h d", h=LH),
                        bv_r,
                    )

                qq = [lambda: q_kq("q", 0), lambda: q_kq("q", 1)]
                kv = [
                    lambda: q_kq("k", 0), lambda: q_kq("k", 1),
                    lambda: q_v(0, 0), lambda: q_v(0, 1),
                    lambda: q_v(1, 0), lambda: q_v(1, 1),
                ]
                return qq, kv

            def make_oproj(qc):
                """4 PE quanta of 2 matmuls each + copy/store per pair."""
                st = {}

                def q_o(a, j):
                    if j == 0:
                        st[a] = scp.tile(
                            [P, 2, QCS], f32, tag="sc", name=f"pso{qc}{a}"
                        )
                    pso = st[a]
                    mo = 2 * a + j
                    msl = slice(mo * P, (mo + 1) * P)
                    for c in range(2):
                        nc.tensor.matmul(
                            pso[:, j, :], wo_sb[:, c, msl], CTs[qc][:, c, :],
                            start=(c == 0), stop=(c == 1),
                        )
                    if j == 1:
                        ost = opool.tile(
                            [P, 2, QCS], bf, tag="ost", name=f"ost{qc}{a}"
                        )
                        nc.vector.tensor_copy(ost[:], pso[:])
                        nc.sync.dma_start(outTt[qc, a], ost[:])

                return [
                    lambda: q_o(0, 0), lambda: q_o(0, 1),
                    lambda: q_o(1, 0), lambda: q_o(1, 1),
                ]

            def attn(qc, pr, early, spread):
                """Attention for (q-chunk qc, head-pair pr). `early` fillers
                are emitted at slots 0..len-1 (deadline-pinned K/V proj);
                `spread` fillers are distributed over the remaining slots."""
                nkc = (qc + 1) * (QCS // KCS)
                fillpos = {}
                for j, f in enumerate(early):
                    fillpos.setdefault(j, []).append(f)
                lo0 = len(early)
                nsl = max(nkc - lo0, 1)
                for j, f in enumerate(spread):
                    pos = lo0 + min(nsl - 1, (j * nsl) // max(len(spread), 1))
                    fillpos.setdefault(pos, []).append(f)

                cpair = cxp.tile([P, 2, QCS], f32, tag="cx", name=f"cp{qc}{pr}")
                pend = []

                def emit_ctx(kc, et):
                    fc, kk = kc // 4, kc % 4
                    lo = max(kc * KCS - qc * QCS, 0)
                    first, last = kc == 0, kc == nkc - 1
                    nc.tensor.matmul(
                        cpair[0 : DK + 1, 0, lo:QCS], Vgs[fc][:, kk, 2 * pr + 0, :],
                        et[:, 0, lo:QCS], start=first, stop=last,
                    )
                    nc.tensor.matmul(
                        cpair[0 : DK + 1, 1, lo:QCS], Vgs[fc][:, kk, 2 * pr + 1, :],
                        et[:, 1, lo:QCS], start=first, stop=last,
                    )

                for kc in range(nkc):
                    fc, kk = kc // 4, kc % 4
                    ksl = slice(kk * KCS, (kk + 1) * KCS)
                    d = kc * KCS - qc * QCS
                    lo = max(d, 0)
                    sct = scp.tile(
                        [P, 2, QCS], f32, tag="sc", name=f"sct{qc}{pr}{kc}"
                    )
                    # two heads of the pair: concurrent row-tiles (0,0)/(64,0)
                    nc.tensor.matmul(
                        sct[:, 0, :], KTs[fc][0:DK, pr, ksl], QTs[qc][0:DK, pr, :],
                        start=True, stop=True,
                    )
                    nc.tensor.matmul(
                        sct[:, 1, :], KTs[fc][DK:P, pr, ksl], QTs[qc][DK:P, pr, :],
                        start=True, stop=True,
                    )
                    if d >= 0:  # diagonal tile: causal window mask
                        nc.vector.tensor_add(
                            sct[:, 0, d : d + P], sct[:, 0, d : d + P], mt_sb[:]
                        )
                        nc.vector.tensor_add(
                            sct[:, 1, d : d + P], sct[:, 1, d : d + P], mt_sb[:]
                        )
                    et = epool.tile([P, 2, QCS], bf, tag="et")
                    nc.scalar.activation(et[:, :, lo:QCS], sct[:, :, lo:QCS], Exp)
                    pend.append((kc, et))
                    if len(pend) > 2:  # ctx trails scores by 2 kc slots
                        emit_ctx(*pend.pop(0))
                    for f in fillpos.get(kc, ()):
                        f()
                for pe_ in pend:
                    emit_ctx(*pe_)

                # softmax denominator tail: l sits on PSUM partition 64 (row 64
                # of the ctx accumulator). Copy both heads' l rows to SBUF
                # (single-bank DVE reads), DMA down to partition 0, fast
                # reciprocal there, GPSIMD broadcast across the 64 ctx
                # partitions, then 2 DVE muls.
                lr = lpool.tile([P, 2, QCS], f32, tag="lr", name=f"lr{qc}{pr}")
                nc.vector.tensor_copy(lr[DK : DK + 1, 0, :], cpair[DK : DK + 1, 0, :])
                nc.vector.tensor_copy(lr[DK : DK + 1, 1, :], cpair[DK : DK + 1, 1, :])
                l0 = lpool.tile([1, 2, QCS], f32, tag="l0", name=f"l0_{qc}{pr}")
                nc.sync.dma_start(l0[:], lr[DK : DK + 1, :, :])
                r0 = lpool.tile([1, 2, QCS], f32, tag="r0", name=f"r0_{qc}{pr}")
                nc.vector.reciprocal_approx_fast(r0[:], l0[:])
                rbA = cbpool.tile([DK, QCS], f32, tag="rbA")
                nc.gpsimd.partition_broadcast(rbA[:], r0[0:1, 0, :], channels=DK)
                rbB = cbpool.tile([DK, QCS], f32, tag="rbB")
                nc.gpsimd.partition_broadcast(rbB[:], r0[0:1, 1, :], channels=DK)
                # head B first: its SBUF hop (partition shift) has the longest
                # path to the consuming output projection
                cbs = cbpool.tile([DK, QCS], bf, tag="cbs")
                nc.vector.tensor_mul(cbs[:], cpair[0:DK, 1, :], rbB[:])
                nc.sync.dma_start(CTs[qc][DK:P, pr, :], cbs[:])
                nc.vector.tensor_mul(CTs[qc][0:DK, pr, :], cpair[0:DK, 0, :], rbA[:])

            # ---- main pipeline ----
            dma_chunk(0)
            dma_weights()
            qq0, kv0 = make_proj(0)
            for qn in qq0 + kv0:
                qn()
            kv_pend = {}
            for i in range(NQC):
                if i + 1 < NQC:
                    dma_chunk(i + 1)
                f0_early = kv_pend.pop(i, [])
                f0_spread = make_oproj(1) if i == 3 else []
                f1_spread = []
                if i + 1 < NQC:
                    qq, kv = make_proj(i + 1)
                    f1_spread += qq
                    kv_pend[i + 1] = kv
                if i == 2:
                    f1_spread += make_oproj(0)
                if i == 3:
                    f1_spread = make_oproj(2)
                attn(i, 0, f0_early, f0_spread)
                attn(i, 1, [], f1_spread)
            for qn in make_oproj(NQC - 1):
                qn()

    nc.compile()
    _CACHE["nc"] = nc
    return nc


def make_in_maps(q, k, v, w_q, b_q, w_k, b_k, w_v, b_v, w_o):
    """Host-side sharding: per-core input dict with pre-tiled DRAM layouts."""
    import ml_dtypes

    f = np.float32
    mmt = ml_dtypes.bfloat16 if MM_DT == "bf16" else f
    q = np.asarray(q, f)
    k = np.asarray(k, f)
    v = np.asarray(v, f)
    w_q = np.asarray(w_q, f)
    w_k = np.asarray(w_k, f)
    w_v = np.asarray(w_v, f)
    w_o = np.asarray(w_o, f)
    b_q = np.asarray(b_q, f)
    b_k = np.asarray(b_k, f)
    b_v = np.asarray(b_v, f)

    scale = np.float32(1.0 / np.sqrt(DK))

    def tile_chunks(xT):
        # [D, S] -> [NQC, P, 4(c), QCS]: per-chunk, partition-major
        return xT.reshape(4, P, NQC, QCS).transpose(2, 1, 0, 3)

    qkvTl = []
    for b in range(B):
        stk = np.stack(
            [tile_chunks(q[b].T), tile_chunks(k[b].T), tile_chunks(v[b].T)],
            axis=2,
        )  # [NQC, P, 3, 4, QCS]
        qkvTl.append(np.ascontiguousarray(stk).astype(mmt))

    ii = np.arange(P)
    mtri = np.where(ii[:, None] > ii[None, :], f(NEG), f(0.0)).astype(f)

    def tile_w(wT):   # [D, DH] -> [P, 4, DH]
        return wT.reshape(4, P, DH).transpose(1, 0, 2)

    per_g = []
    for g in range(HG):
        gsl = slice(g * DH, (g + 1) * DH)
        wqkv = np.stack(
            [
                tile_w(w_q[gsl, :].T),
                tile_w(w_k[gsl, :].T * scale),
                tile_w(w_v[gsl, :].T),
            ],
            axis=1,
        )  # [P, 3, 4, DH]
        woTt = w_o[:, gsl].T.reshape(2, P, D).transpose(1, 0, 2)  # [P, 2, D]
        bqk = np.stack(
            [b_q[gsl].reshape(2, P).T, (b_k[gsl] * scale).reshape(2, P).T],
            axis=1,
        )  # [P, 2(q/k), 2(mo)]
        per_g.append(
            dict(
                wqkvT=np.ascontiguousarray(wqkv).astype(mmt),
                woTt=np.ascontiguousarray(woTt).astype(mmt),
                bqk=np.ascontiguousarray(bqk),
                bv=np.ascontiguousarray(np.broadcast_to(b_v[gsl], (P, DH))),
                mtri=mtri,
                onesd=np.ones((P, DK), mmt),
            )
        )

    in_maps = []
    for c in range(NCORES):
        b, g = c // HG, c % HG
        m = dict(qkvT=qkvTl[b], **per_g[g])
        in_maps.append(m)
    return in_maps


def gather(results, b_o):
    """Sum head-group partials per batch, un-tile, un-transpose, add b_o."""
    b_o = np.asarray(b_o, np.float32)
    out = np.empty((B, S, D), np.float32)
    for b in range(B):
        acc = results[HG * b]["outTt"].astype(np.float32) + results[
            HG * b + 1
        ]["outTt"].astype(np.float32)
        # [NQC, 2(a), P, 2(j), QCS] -> [D, S]
        full = acc.transpose(1, 3, 2, 0, 4).reshape(D, S)
        out[b] = full.T + b_o
    return out


def kernel(q, k, v, mask, w_q, b_q, w_k, b_k, w_v, b_v, w_o, b_o, **run_kwargs):
    _import_concourse()
    from concourse.bass_utils import run_bass_kernel_spmd

    nc = build_nc()
    in_maps = make_in_maps(q, k, v, w_q, b_q, w_k, b_k, w_v, b_v, w_o)
    res = run_bass_kernel_spmd(nc, in_maps, core_ids=list(range(NCORES)), **run_kwargs)
    out = gather(res.results, b_o)
    kernel.last_result = res
    return out
